# revision 19
# baseline (speedup 1.0000x reference)
"""DeepFM (nn_DeepFM_77558519431939) Trainium2 Bass kernel.

Strategy (8 NeuronCores, SPMD, no collectives):
  - Replicate the embedding table on every core; data-parallel the batch
    (16384 samples -> 2048 per core).  Each gathered row is fetched exactly
    once across the fleet, and there is no all-to-all.
  - The table is stored in bf16 [S, 12]: 10 embedding dims, w_first value
    (first-order weight) in col 10, zero pad in col 11.  Rows are gathered
    with indirect DMAs (the HW contract on this image is one offset per
    partition per instruction = 128 rows each); all 128 instructions are
    issued dependency-free back-to-back across two SWDGE queues so the
    GpSimd engine streams at full occupancy, and the whole compute
    pipeline hides underneath the gather stream.
  - Gathered rows land sample-on-partition; PE transposes flip them into a
    feature-major activation matrix X [104, 2048] (bf16):
        rows f*12+e (e<10): emb dim e of field f
        rows f*12+10:       w_first value of field f
        rows f*12+11:       zero pad
        rows 96..102:       raw dense features (transposed on host)
        row 103:            constant 1.0 (bias row)
  - The whole DeepFM head runs on 512-wide column tiles in bf16 (PSUM
    accumulation in fp32) with all the small weights folded on the host:
        H1 = relu(W1s^T X)            (dense-proj + b1 folded into W1s)
        H2 = relu(W2^T H1 + b2)
        SD = sdw^T X                  (rows 0..9 = s, 10..19 = dense_emb,
                                       row 20 = first-order linear term)
        Z  = [X[0:96]^2 ; SD[0:20]^2 ; SD[20]]
        FIN = zw^T Z + W3^T H2        (zw = +-0.5 masks + lin passthrough)
        out = sigmoid(FIN)
"""

import os
from contextlib import ExitStack

import numpy as np
import ml_dtypes

import concourse.bass as bass
import concourse.bacc as bacc
import concourse.mybir as mybir
import concourse.tile as tile

# ---- problem constants (hardcoded; must match the reference) ----
VOCABS = [1000000, 500000, 200000, 100000, 50000, 10000, 5000, 1000]
S = int(np.sum(VOCABS))  # 1,866,000
OFFSETS = np.concatenate([[0], np.cumsum(VOCABS)[:-1]]).astype(np.int64)
B = 16384
EMB = 10
N_DENSE = 7
F = len(VOCABS)  # 8
HID = 128

N_CORES = 8
BL = B // N_CORES  # 2048 per core
RW = 12            # augmented table row width (10 emb + wf + pad)
KX = 104           # X partition rows: 96 gathered + 7 dense + 1 const
NZ = 116           # Z rows: 96 emb^2 + 20 sd^2
NBLK = BL // 128   # 16 sample blocks of 128
NGRP = 4           # column-tile groups
GBLK = NBLK // NGRP  # 4 blocks of 128 samples per group
TW = 128 * GBLK    # column-tile width = 512

F32 = mybir.dt.float32
BF16 = mybir.dt.bfloat16
I32 = mybir.dt.int32
BF = ml_dtypes.bfloat16

_cached = {}


def _build_program(debug_dump=False):
    """Build the SPMD Bass program (same for all cores)."""
    nc = bacc.Bacc("TRN2", target_bir_lowering=False, debug=False,
                   num_swdge_queues=2)

    tab_d = nc.dram_tensor("tab", [S, RW], BF16, kind="ExternalInput").ap()
    idx_d = nc.dram_tensor("idxs", [128, 128], I32, kind="ExternalInput").ap()
    dn8_d = nc.dram_tensor("dn8", [8, BL], BF16, kind="ExternalInput").ap()
    # all small weights packed into one bf16 tensor: one DMA, one sem wait
    # cols: idn 0:128 | w1s 128:256 | w2 256:384 | sdw 384:404 | zw 404 |
    #       w3 405 | a1 406
    wpk_d = nc.dram_tensor("wpk", [128, 407], BF16, kind="ExternalInput").ap()
    b2_d = nc.dram_tensor("b2v", [128, 1], F32, kind="ExternalInput").ap()
    out_d = nc.dram_tensor("out", [1, BL], F32, kind="ExternalOutput").ap()
    if debug_dump:
        xdmp_d = nc.dram_tensor("xdmp", [KX, BL], F32, kind="ExternalOutput").ap()
        fdmp_d = nc.dram_tensor("fdmp", [1, BL], F32, kind="ExternalOutput").ap()

    with ExitStack() as ctx:
        tc = ctx.enter_context(tile.TileContext(nc))
        const = ctx.enter_context(tc.tile_pool(name="const", bufs=1))
        gpool = ctx.enter_context(tc.tile_pool(name="gch", bufs=NBLK))
        hpool = ctx.enter_context(tc.tile_pool(name="h", bufs=4))
        qpool = ctx.enter_context(tc.tile_pool(name="z", bufs=2))
        pp_x = ctx.enter_context(tc.tile_pool(name="ppx", bufs=2, space="PSUM"))
        pp_h = ctx.enter_context(tc.tile_pool(name="pph", bufs=2, space="PSUM"))
        pp_s = ctx.enter_context(tc.tile_pool(name="pps", bufs=2, space="PSUM"))
        pp_f = ctx.enter_context(tc.tile_pool(name="ppf", bufs=1, space="PSUM"))

        # index tile first: the gathers depend only on it; split the load so
        # block 0's gathers wait on a 4KB DMA, not the full 64KB
        idx_t = const.tile([128, 128], I32)
        nc.sync.dma_start(idx_t[:, 0:F], idx_d[:, 0:F])
        nc.sync.dma_start(idx_t[:, F:128], idx_d[:, F:128])

        # constants: one packed tile, sliced below
        wpk_t = const.tile([128, 407], BF16)
        nc.sync.dma_start(wpk_t[:], wpk_d[:])
        b2_t = const.tile([128, 1], F32)
        nc.sync.dma_start(b2_t[:], b2_d[:])
        idn_t = wpk_t[:, 0:128]
        w1s_t = wpk_t[0:KX, 128:256]
        w2_t = wpk_t[:, 256:384]
        sdw_t = wpk_t[0:KX, 384:404]
        zw_t = wpk_t[0:NZ, 404:405]
        w3_t = wpk_t[:, 405:406]
        a1_t = wpk_t[0:KX, 406:407]

        # X: feature-major activations (bf16)
        x_t = const.tile([KX, BL], BF16)
        nc.sync.dma_start(x_t[96:104, :], dn8_d[:])

        out_sb = const.tile([1, BL], F32)
        if debug_dump:
            fin_sb = const.tile([1, BL], F32)

        RELU = mybir.ActivationFunctionType.Relu
        SIGMOID = mybir.ActivationFunctionType.Sigmoid
        SQUARE = mybir.ActivationFunctionType.Square

        # gathers: HW indirect DMA moves one row per partition per
        # instruction (128 offsets); issue all 128 back-to-back so the
        # GpSimd engine never stalls and compute pipelines underneath.
        gbs = []
        for j in range(NBLK):
            gb = gpool.tile([128, F * RW], BF16, tag="gch")
            for f in range(F):
                gi = nc.gpsimd.indirect_dma_start(
                    out=gb[:, f * RW:(f + 1) * RW],
                    out_offset=None,
                    in_=tab_d[:],
                    in_offset=bass.IndirectOffsetOnAxis(
                        ap=idx_t[:, j * F + f:j * F + f + 1], axis=0
                    ),
                )
                # alternate SWDGE queues so ring drain never backpressures
                if (j * F + f) % 2 == 1:
                    gi.ins.queue = "qPoolDynamic1"
            gbs.append(gb)

        for g in range(NGRP):
            cols = slice(TW * g, TW * (g + 1))
            xp = pp_x.tile([96, TW], BF16, tag="xp")
            h1p = pp_h.tile([HID, TW], F32, tag="hp")
            h1_t = hpool.tile([HID, TW], BF16, tag="h")
            h2p = pp_h.tile([HID, TW], F32, tag="hp")
            h2_t = hpool.tile([HID, TW], BF16, tag="h")
            sdp = pp_s.tile([20, TW], F32, tag="sd")
            z_t = qpool.tile([NZ, TW], BF16, tag="z")
            fin = pp_f.tile([1, TW], F32, tag="fin")

            # the last group runs per-128-column pipelines (column slices of
            # the same tiles) so the chain after the final gather is short;
            # earlier groups use wide ops (fewer instructions, fully hidden
            # under the gather stream)
            subs = ([slice(128 * b, 128 * (b + 1)) for b in range(GBLK)]
                    if g == NGRP - 1 else [slice(0, TW)])
            for b in range(GBLK):
                nc.tensor.transpose(
                    out=xp[:, 128 * b:128 * (b + 1)],
                    in_=gbs[g * GBLK + b][:],
                    identity=idn_t,
                )
                if g == NGRP - 1:
                    sc = slice(TW * g + 128 * b, TW * g + 128 * (b + 1))
                    nc.vector.tensor_copy(x_t[0:96, sc],
                                          xp[:, 128 * b:128 * (b + 1)])
            if g != NGRP - 1:
                nc.vector.tensor_copy(x_t[0:96, cols], xp[:])

            for sb in subs:
                gc = slice(TW * g + sb.start, TW * g + sb.stop)
                nc.tensor.matmul(out=h1p[:, sb], lhsT=w1s_t, rhs=x_t[:, gc],
                                 start=True, stop=True)
                nc.vector.tensor_scalar_max(h1_t[:, sb], h1p[:, sb], 0.0)
                nc.tensor.matmul(out=h2p[:, sb], lhsT=w2_t, rhs=h1_t[:, sb],
                                 start=True, stop=True)
                nc.scalar.activation(h2_t[:, sb], h2p[:, sb], RELU, bias=b2_t)
                nc.tensor.matmul(out=sdp[:, sb], lhsT=sdw_t, rhs=x_t[:, gc],
                                 start=True, stop=True)
                nc.vector.tensor_mul(z_t[0:96, sb], x_t[0:96, gc],
                                     x_t[0:96, gc])
                nc.scalar.activation(z_t[96:116, sb], sdp[0:20, sb], SQUARE)
                nc.tensor.matmul(out=fin[:, sb], lhsT=a1_t, rhs=x_t[:, gc],
                                 start=True, stop=False)
                nc.tensor.matmul(out=fin[:, sb], lhsT=zw_t, rhs=z_t[:, sb],
                                 start=False, stop=False)
                nc.tensor.matmul(out=fin[:, sb], lhsT=w3_t, rhs=h2_t[:, sb],
                                 start=False, stop=True)
                if debug_dump:
                    nc.vector.tensor_copy(fin_sb[:, gc], fin[:, sb])
                nc.scalar.activation(out_sb[:, gc], fin[:, sb], SIGMOID)
                if g == NGRP - 1:
                    nc.sync.dma_start(out_d[:, gc], out_sb[:, gc])
            if g != NGRP - 1:
                nc.sync.dma_start(out_d[:, cols], out_sb[:, cols])
        if debug_dump:
            xdmp_t = const.tile([KX, BL], F32)
            nc.vector.tensor_copy(xdmp_t[:], x_t[:])
            nc.sync.dma_start(xdmp_d[:], xdmp_t[:])
            nc.sync.dma_start(fdmp_d[:], fin_sb[:])

    nc.compile()
    return nc


def _host_prep(sparse_feature, dense_feature, emb_table, W_dense, b_dense,
               w_first, b_first, W1, b1, W2, b2, W3, b3):
    """Build the augmented table, folded weights, and per-core in_maps."""
    f32 = np.float32
    emb_table = np.asarray(emb_table, dtype=f32)
    W_dense = np.asarray(W_dense, dtype=f32)      # [10, 7]
    b_dense = np.asarray(b_dense, dtype=f32)      # [10]
    w_first = np.asarray(w_first, dtype=f32)      # [S+7]
    b_first = np.asarray(b_first, dtype=f32)      # [1]
    W1 = np.asarray(W1, dtype=f32)                # [90, 128]
    b1 = np.asarray(b1, dtype=f32)                # [128]
    W2 = np.asarray(W2, dtype=f32)                # [128, 128]
    b2 = np.asarray(b2, dtype=f32)                # [128]
    W3 = np.asarray(W3, dtype=f32)                # [128, 1]
    b3 = np.asarray(b3, dtype=f32)                # [1]

    tab = np.zeros((S, RW), dtype=f32)
    tab[:, :EMB] = emb_table
    tab[:, EMB] = w_first[:S]

    w1s = np.zeros((KX, HID), dtype=f32)
    for f in range(F):
        w1s[f * RW:f * RW + EMB] = W1[f * EMB:(f + 1) * EMB]
    w1s[96:103] = W_dense.T @ W1[F * EMB:]               # [7,128]
    w1s[103] = b1 + b_dense @ W1[F * EMB:]

    # sdw: cols 0..9 = s, 10..19 = dense_emb
    sdw = np.zeros((KX, 20), dtype=f32)
    for f in range(F):
        for e in range(EMB):
            sdw[f * RW + e, e] = 1.0
    sdw[96:103, 0:10] = W_dense.T
    sdw[103, 0:10] = b_dense
    sdw[96:103, 10:20] = W_dense.T
    sdw[103, 10:20] = b_dense

    # a1: first-order linear term (gathered w_first rows + dense + biases)
    a1 = np.zeros((KX, 1), dtype=f32)
    for f in range(F):
        a1[f * RW + EMB] = 1.0
    a1[96:103, 0] = w_first[S:]
    a1[103] = b_first[0] + b3[0]

    # zw over Z = [x^2 (96) ; s^2/demb^2 (20) ; lin (1)]
    zw = np.zeros((NZ, 1), dtype=f32)
    for f in range(F):
        zw[f * RW:f * RW + EMB] = -0.5
    zw[96:106] = 0.5
    zw[106:116] = -0.5

    idx_g = (np.asarray(sparse_feature, dtype=np.int64)
             + OFFSETS[None, :]).astype(np.int32)         # [B, F]
    dense = np.asarray(dense_feature, dtype=f32)          # [B, 7]

    wpk = np.zeros((128, 407), dtype=f32)
    wpk[:, 0:128] = np.eye(128, dtype=f32)
    wpk[0:KX, 128:256] = w1s
    wpk[:, 256:384] = W2
    wpk[0:KX, 384:404] = sdw
    wpk[0:NZ, 404] = zw[:, 0]
    wpk[:, 405] = W3.reshape(HID)
    wpk[0:KX, 406] = a1[:, 0]

    common = {"tab": tab.astype(BF), "wpk": wpk.astype(BF),
              "b2v": b2.reshape(128, 1).copy()}
    in_maps = []
    for c in range(N_CORES):
        lo, hi = c * BL, (c + 1) * BL
        lg = idx_g[lo:hi].reshape(NBLK, 128, F)
        idxs = np.ascontiguousarray(
            lg.transpose(1, 0, 2).reshape(128, NBLK * F))  # [128, 128]
        dn8 = np.ones((8, BL), dtype=f32)
        dn8[:7] = dense[lo:hi].T
        in_maps.append(dict(common, idxs=idxs, dn8=dn8.astype(BF)))
    return in_maps


def _get_program(debug_dump=False):
    key = ("nc", debug_dump)
    if key not in _cached:
        _cached[key] = _build_program(debug_dump)
    return _cached[key]


def run_on_device(in_maps, trace=False, debug_dump=False):
    """Run the SPMD program on 8 NeuronCores.  Returns (results, exec_time_ns)."""
    from concourse.bass_utils import run_bass_kernel_spmd

    nc = _get_program(debug_dump)
    res = run_bass_kernel_spmd(nc, in_maps, list(range(N_CORES)), trace=trace)
    return res.results, res.exec_time_ns


def kernel(**inputs):
    in_maps = _host_prep(**inputs)
    results, _ = run_on_device(in_maps, trace=False)
    out = np.concatenate([results[c]["out"].reshape(BL) for c in range(N_CORES)])
    return out.astype(np.float32)


# revision 20
# speedup vs baseline: 1.2897x; 1.2897x over previous
"""DeepFM (nn_DeepFM_77558519431939) Trainium2 Bass kernel.

Strategy (8 NeuronCores, SPMD, no collectives):
  - Replicate the embedding table on every core; data-parallel the batch
    (16384 samples -> 2048 per core).  Each gathered row is fetched exactly
    once across the fleet, and there is no all-to-all.
  - The table is stored in bf16 [S, 12]: 10 embedding dims, w_first value
    (first-order weight) in col 10, zero pad in col 11.  Rows are gathered
    with indirect DMAs (the HW contract on this image is one offset per
    partition per instruction = 128 rows each); all 128 instructions are
    issued dependency-free back-to-back across two SWDGE queues so the
    GpSimd engine streams at full occupancy, and the whole compute
    pipeline hides underneath the gather stream.
  - Gathered rows land sample-on-partition; PE transposes flip them into a
    feature-major activation matrix X [104, 2048] (bf16):
        rows f*12+e (e<10): emb dim e of field f
        rows f*12+10:       w_first value of field f
        rows f*12+11:       zero pad
        rows 96..102:       raw dense features (transposed on host)
        row 103:            constant 1.0 (bias row)
  - The whole DeepFM head runs on 512-wide column tiles in bf16 (PSUM
    accumulation in fp32) with all the small weights folded on the host:
        H1 = relu(W1s^T X)            (dense-proj + b1 folded into W1s)
        H2 = relu(W2^T H1 + b2)
        SD = sdw^T X                  (rows 0..9 = s, 10..19 = dense_emb,
                                       row 20 = first-order linear term)
        Z  = [X[0:96]^2 ; SD[0:20]^2 ; SD[20]]
        FIN = zw^T Z + W3^T H2        (zw = +-0.5 masks + lin passthrough)
        out = sigmoid(FIN)
"""

import os
from contextlib import ExitStack

import numpy as np
import ml_dtypes

import concourse.bass as bass
import concourse.bacc as bacc
import concourse.mybir as mybir
import concourse.tile as tile

# ---- problem constants (hardcoded; must match the reference) ----
VOCABS = [1000000, 500000, 200000, 100000, 50000, 10000, 5000, 1000]
S = int(np.sum(VOCABS))  # 1,866,000
OFFSETS = np.concatenate([[0], np.cumsum(VOCABS)[:-1]]).astype(np.int64)
B = 16384
EMB = 10
N_DENSE = 7
F = len(VOCABS)  # 8
HID = 128

N_CORES = 8
BL = B // N_CORES  # 2048 per core
RW = 12            # augmented table row width (10 emb + wf + pad)
KX = 104           # X partition rows: 96 gathered + 7 dense + 1 const
NZ = 116           # Z rows: 96 emb^2 + 20 sd^2
NBLK = BL // 128   # 16 sample blocks of 128
NGRP = 4           # column-tile groups
GBLK = NBLK // NGRP  # 4 blocks of 128 samples per group
TW = 128 * GBLK    # column-tile width = 512

F32 = mybir.dt.float32
BF16 = mybir.dt.bfloat16
I32 = mybir.dt.int32
BF = ml_dtypes.bfloat16

_cached = {}


def _build_program(debug_dump=False):
    """Build the SPMD Bass program (same for all cores)."""
    nc = bacc.Bacc("TRN2", target_bir_lowering=False, debug=False,
                   num_swdge_queues=2)

    tab_d = nc.dram_tensor("tab", [S, RW], BF16, kind="ExternalInput").ap()
    t67_d = nc.dram_tensor("t67", [VOCABS[6] * VOCABS[7], 2 * RW], BF16,
                           kind="ExternalInput").ap()
    idx_d = nc.dram_tensor("idxs", [128, 128], I32, kind="ExternalInput").ap()
    dn8_d = nc.dram_tensor("dn8", [8, BL], BF16, kind="ExternalInput").ap()
    # all small weights packed into one bf16 tensor: one DMA, one sem wait
    # cols: idn 0:128 | w1s 128:256 | w2 256:384 | sdw 384:404 | zw 404 |
    #       w3 405 | a1 406
    wpk_d = nc.dram_tensor("wpk", [128, 407], BF16, kind="ExternalInput").ap()
    b2_d = nc.dram_tensor("b2v", [128, 1], F32, kind="ExternalInput").ap()
    out_d = nc.dram_tensor("out", [1, BL], F32, kind="ExternalOutput").ap()
    if debug_dump:
        xdmp_d = nc.dram_tensor("xdmp", [KX, BL], F32, kind="ExternalOutput").ap()
        fdmp_d = nc.dram_tensor("fdmp", [1, BL], F32, kind="ExternalOutput").ap()

    with ExitStack() as ctx:
        tc = ctx.enter_context(tile.TileContext(nc))
        const = ctx.enter_context(tc.tile_pool(name="const", bufs=1))
        gpool = ctx.enter_context(tc.tile_pool(name="gch", bufs=NBLK))
        hpool = ctx.enter_context(tc.tile_pool(name="h", bufs=4))
        qpool = ctx.enter_context(tc.tile_pool(name="z", bufs=2))
        pp_x = ctx.enter_context(tc.tile_pool(name="ppx", bufs=2, space="PSUM"))
        pp_h = ctx.enter_context(tc.tile_pool(name="pph", bufs=2, space="PSUM"))
        pp_s = ctx.enter_context(tc.tile_pool(name="pps", bufs=2, space="PSUM"))
        pp_f = ctx.enter_context(tc.tile_pool(name="ppf", bufs=1, space="PSUM"))

        # index tile first: the gathers depend only on it; split the load so
        # block 0's gathers wait on a 4KB DMA, not the full 64KB
        idx_t = const.tile([128, 128], I32)
        nc.sync.dma_start(idx_t[:, 0:F], idx_d[:, 0:F])
        nc.sync.dma_start(idx_t[:, F:128], idx_d[:, F:128])

        # constants: one packed tile, sliced below
        wpk_t = const.tile([128, 407], BF16)
        nc.sync.dma_start(wpk_t[:], wpk_d[:])
        b2_t = const.tile([128, 1], F32)
        nc.sync.dma_start(b2_t[:], b2_d[:])
        idn_t = wpk_t[:, 0:128]
        w1s_t = wpk_t[0:KX, 128:256]
        w2_t = wpk_t[:, 256:384]
        sdw_t = wpk_t[0:KX, 384:404]
        zw_t = wpk_t[0:NZ, 404:405]
        w3_t = wpk_t[:, 405:406]
        a1_t = wpk_t[0:KX, 406:407]

        # X: feature-major activations (bf16)
        x_t = const.tile([KX, BL], BF16)
        nc.sync.dma_start(x_t[96:104, :], dn8_d[:])

        out_sb = const.tile([1, BL], F32)
        if debug_dump:
            fin_sb = const.tile([1, BL], F32)

        RELU = mybir.ActivationFunctionType.Relu
        SIGMOID = mybir.ActivationFunctionType.Sigmoid
        SQUARE = mybir.ActivationFunctionType.Square

        # gathers: HW indirect DMA moves one row per partition per
        # instruction (128 offsets); issue all 128 back-to-back so the
        # GpSimd engine never stalls and compute pipelines underneath.
        gbs = []
        for j in range(NBLK):
            gb = gpool.tile([128, F * RW], BF16, tag="gch")
            for f in range(7):
                gi = nc.gpsimd.indirect_dma_start(
                    out=(gb[:, f * RW:(f + 1) * RW] if f < 6
                         else gb[:, 6 * RW:8 * RW]),
                    out_offset=None,
                    in_=(tab_d[:] if f < 6 else t67_d[:]),
                    in_offset=bass.IndirectOffsetOnAxis(
                        ap=idx_t[:, j * F + f:j * F + f + 1], axis=0
                    ),
                )
                if (j * 7 + f) % 2 == 1:
                    gi.ins.queue = "qPoolDynamic1"
            gbs.append(gb)

        for g in range(NGRP):
            cols = slice(TW * g, TW * (g + 1))
            xp = pp_x.tile([96, TW], BF16, tag="xp")
            h1p = pp_h.tile([HID, TW], F32, tag="hp")
            h1_t = hpool.tile([HID, TW], BF16, tag="h")
            h2p = pp_h.tile([HID, TW], F32, tag="hp")
            h2_t = hpool.tile([HID, TW], BF16, tag="h")
            sdp = pp_s.tile([20, TW], F32, tag="sd")
            z_t = qpool.tile([NZ, TW], BF16, tag="z")
            fin = pp_f.tile([1, TW], F32, tag="fin")

            # the last group runs per-128-column pipelines (column slices of
            # the same tiles) so the chain after the final gather is short;
            # earlier groups use wide ops (fewer instructions, fully hidden
            # under the gather stream)
            subs = ([slice(128 * b, 128 * (b + 1)) for b in range(GBLK)]
                    if g == NGRP - 1 else [slice(0, TW)])
            for b in range(GBLK):
                nc.tensor.transpose(
                    out=xp[:, 128 * b:128 * (b + 1)],
                    in_=gbs[g * GBLK + b][:],
                    identity=idn_t,
                )
                if g == NGRP - 1:
                    sc = slice(TW * g + 128 * b, TW * g + 128 * (b + 1))
                    nc.vector.tensor_copy(x_t[0:96, sc],
                                          xp[:, 128 * b:128 * (b + 1)])
            if g != NGRP - 1:
                nc.vector.tensor_copy(x_t[0:96, cols], xp[:])

            for sb in subs:
                gc = slice(TW * g + sb.start, TW * g + sb.stop)
                nc.tensor.matmul(out=h1p[:, sb], lhsT=w1s_t, rhs=x_t[:, gc],
                                 start=True, stop=True)
                nc.vector.tensor_scalar_max(h1_t[:, sb], h1p[:, sb], 0.0)
                nc.tensor.matmul(out=h2p[:, sb], lhsT=w2_t, rhs=h1_t[:, sb],
                                 start=True, stop=True)
                nc.scalar.activation(h2_t[:, sb], h2p[:, sb], RELU, bias=b2_t)
                nc.tensor.matmul(out=sdp[:, sb], lhsT=sdw_t, rhs=x_t[:, gc],
                                 start=True, stop=True)
                nc.vector.tensor_mul(z_t[0:96, sb], x_t[0:96, gc],
                                     x_t[0:96, gc])
                nc.scalar.activation(z_t[96:116, sb], sdp[0:20, sb], SQUARE)
                nc.tensor.matmul(out=fin[:, sb], lhsT=a1_t, rhs=x_t[:, gc],
                                 start=True, stop=False)
                nc.tensor.matmul(out=fin[:, sb], lhsT=zw_t, rhs=z_t[:, sb],
                                 start=False, stop=False)
                nc.tensor.matmul(out=fin[:, sb], lhsT=w3_t, rhs=h2_t[:, sb],
                                 start=False, stop=True)
                if debug_dump:
                    nc.vector.tensor_copy(fin_sb[:, gc], fin[:, sb])
                nc.scalar.activation(out_sb[:, gc], fin[:, sb], SIGMOID)
                if g == NGRP - 1:
                    nc.sync.dma_start(out_d[:, gc], out_sb[:, gc])
            if g != NGRP - 1:
                nc.sync.dma_start(out_d[:, cols], out_sb[:, cols])
        if debug_dump:
            xdmp_t = const.tile([KX, BL], F32)
            nc.vector.tensor_copy(xdmp_t[:], x_t[:])
            nc.sync.dma_start(xdmp_d[:], xdmp_t[:])
            nc.sync.dma_start(fdmp_d[:], fin_sb[:])

    nc.compile()
    return nc


def _host_prep(sparse_feature, dense_feature, emb_table, W_dense, b_dense,
               w_first, b_first, W1, b1, W2, b2, W3, b3):
    """Build the augmented table, folded weights, and per-core in_maps."""
    f32 = np.float32
    emb_table = np.asarray(emb_table, dtype=f32)
    W_dense = np.asarray(W_dense, dtype=f32)      # [10, 7]
    b_dense = np.asarray(b_dense, dtype=f32)      # [10]
    w_first = np.asarray(w_first, dtype=f32)      # [S+7]
    b_first = np.asarray(b_first, dtype=f32)      # [1]
    W1 = np.asarray(W1, dtype=f32)                # [90, 128]
    b1 = np.asarray(b1, dtype=f32)                # [128]
    W2 = np.asarray(W2, dtype=f32)                # [128, 128]
    b2 = np.asarray(b2, dtype=f32)                # [128]
    W3 = np.asarray(W3, dtype=f32)                # [128, 1]
    b3 = np.asarray(b3, dtype=f32)                # [1]

    tab = np.zeros((S, RW), dtype=f32)
    tab[:, :EMB] = emb_table
    tab[:, EMB] = w_first[:S]

    w1s = np.zeros((KX, HID), dtype=f32)
    for f in range(F):
        w1s[f * RW:f * RW + EMB] = W1[f * EMB:(f + 1) * EMB]
    w1s[96:103] = W_dense.T @ W1[F * EMB:]               # [7,128]
    w1s[103] = b1 + b_dense @ W1[F * EMB:]

    # sdw: cols 0..9 = s, 10..19 = dense_emb
    sdw = np.zeros((KX, 20), dtype=f32)
    for f in range(F):
        for e in range(EMB):
            sdw[f * RW + e, e] = 1.0
    sdw[96:103, 0:10] = W_dense.T
    sdw[103, 0:10] = b_dense
    sdw[96:103, 10:20] = W_dense.T
    sdw[103, 10:20] = b_dense

    # a1: first-order linear term (gathered w_first rows + dense + biases)
    a1 = np.zeros((KX, 1), dtype=f32)
    for f in range(F):
        a1[f * RW + EMB] = 1.0
    a1[96:103, 0] = w_first[S:]
    a1[103] = b_first[0] + b3[0]

    # zw over Z = [x^2 (96) ; s^2/demb^2 (20) ; lin (1)]
    zw = np.zeros((NZ, 1), dtype=f32)
    for f in range(F):
        zw[f * RW:f * RW + EMB] = -0.5
    zw[96:106] = 0.5
    zw[106:116] = -0.5

    idx_g = (np.asarray(sparse_feature, dtype=np.int64)
             + OFFSETS[None, :]).astype(np.int32)         # [B, F]
    sp = np.asarray(sparse_feature, dtype=np.int64)
    idx_g[:, 6] = (sp[:, 6] * VOCABS[7] + sp[:, 7]).astype(np.int32)
    idx_g[:, 7] = 0
    v6, v7 = VOCABS[6], VOCABS[7]
    o6, o7 = OFFSETS[6], OFFSETS[7]
    t67 = np.zeros((v6, v7, 2 * RW), dtype=f32)
    t67[:, :, 0:EMB] = emb_table[o6:o6 + v6, None, :]
    t67[:, :, EMB] = w_first[o6:o6 + v6, None]
    t67[:, :, RW:RW + EMB] = emb_table[None, o7:o7 + v7, :]
    t67[:, :, RW + EMB] = w_first[None, o7:o7 + v7]
    t67 = t67.reshape(v6 * v7, 2 * RW)
    dense = np.asarray(dense_feature, dtype=f32)          # [B, 7]

    wpk = np.zeros((128, 407), dtype=f32)
    wpk[:, 0:128] = np.eye(128, dtype=f32)
    wpk[0:KX, 128:256] = w1s
    wpk[:, 256:384] = W2
    wpk[0:KX, 384:404] = sdw
    wpk[0:NZ, 404] = zw[:, 0]
    wpk[:, 405] = W3.reshape(HID)
    wpk[0:KX, 406] = a1[:, 0]

    common = {"tab": tab.astype(BF), "t67": t67.astype(BF),
              "wpk": wpk.astype(BF), "b2v": b2.reshape(128, 1).copy()}
    in_maps = []
    for c in range(N_CORES):
        lo, hi = c * BL, (c + 1) * BL
        lg = idx_g[lo:hi].reshape(NBLK, 128, F)
        idxs = np.ascontiguousarray(
            lg.transpose(1, 0, 2).reshape(128, NBLK * F))  # [128, 128]
        dn8 = np.ones((8, BL), dtype=f32)
        dn8[:7] = dense[lo:hi].T
        in_maps.append(dict(common, idxs=idxs, dn8=dn8.astype(BF)))
    return in_maps


def _get_program(debug_dump=False):
    key = ("nc", debug_dump)
    if key not in _cached:
        _cached[key] = _build_program(debug_dump)
    return _cached[key]


def run_on_device(in_maps, trace=False, debug_dump=False):
    """Run the SPMD program on 8 NeuronCores.  Returns (results, exec_time_ns)."""
    from concourse.bass_utils import run_bass_kernel_spmd

    nc = _get_program(debug_dump)
    res = run_bass_kernel_spmd(nc, in_maps, list(range(N_CORES)), trace=trace)
    return res.results, res.exec_time_ns


def kernel(**inputs):
    in_maps = _host_prep(**inputs)
    results, _ = run_on_device(in_maps, trace=False)
    out = np.concatenate([results[c]["out"].reshape(BL) for c in range(N_CORES)])
    return out.astype(np.float32)
